# revision 20
# baseline (speedup 1.0000x reference)
"""Trainium2 Bass kernel for nn_DecomposableAttentionModel.

Math: the reference's softmax is over a size-1 axis, so attn == 1 exactly and

    out[b] = S[b] @ (W2 @ Wo) + N*L*(b2 @ Wo) + bo
    S[b,h] = sum_{n,l} relu(X[b,n,h] + Y[b,l,h])
    X      = gnn(adj, nodes) @ W1[:CG]
    Y      = prot @ (Wr @ W1[CG:]) + (br @ W1[CG:]) + b1

The [B, N*L, CG+CP] concat tensor is never materialized. Since
w*relu(z) = sign(w)*relu(|w|*z), |w2o| is folded into X and Y at PSUM
evacuation (free per-partition scale on the ACT copy) and the PE reduce
uses exact sign(w2o) stationary vectors.

Sharding: data-parallel over batch B=8, one graph per NeuronCore; weights
replicated. No collectives; the host assembles the [8,1] output. All
weights/biases ship as one packed [128, WCOLS] blob -> a single DMA.

Per-core hot loop (192 units: 128 chunk1 n's + 64 packed chunk2 pairs):
every unit is one relu pass (DVE tensor_scalar 2x | ACT activation) of
Y'[128,512]+x'_u into a wide bf16 buffer; PE folds each slot into
ps_red[1,512] via sign(w2o)^T @ slice matmuls accumulated in PSUM.
"""

import sys

if "/opt/trn_rl_repo" not in sys.path:
    sys.path.insert(0, "/opt/trn_rl_repo")

import numpy as np

import concourse.bacc as bacc
import concourse.bass as bass
import concourse.mybir as mybir
import concourse.tile as tile
from concourse.bass_utils import run_bass_kernel_spmd

B, N, NODE_DIM = 8, 128, 64
L, RES_DIM = 512, 1024
CG, CP = 128, 128
HID = CG + CP // 2  # 192
NUM_GNN_STEPS = 3
NCORES = 8

F32 = mybir.dt.float32
BF16 = mybir.dt.bfloat16
F8 = mybir.dt.float8e4
AF = mybir.ActivationFunctionType
ALU = mybir.AluOpType

# fraction of fused-loop units on the scalar (ACT) engine; rest on DVE.
ACT_FRAC = 0.30
G = 4  # wide-buffer slots per flush
SCALE8 = 256.0  # keeps |w2o|-scaled relu values in fp8e4 range

# packed weight-blob column layout: name -> (col_offset, cols)
_WB = {}
_c = 0
for _name, _w in [
    # part 1 (GNN-critical, lands first): adj..rowvec
    ("adj", N), ("eye", N), ("Wg", CG), ("Wn", N), ("nodesT", N),
    ("W1t", HID), ("W2T", HID), ("cols3", 3), ("rowvec", 449),
    # part 2: protein-path weights
    ("WrT", RES_DIM), ("W1b", HID),
]:
    _WB[_name] = (_c, _w)
    _c += _w
WCOLS = _c
WSPLIT = _WB["WrT"][0]  # columns before this land in the first wblob DMA

_CACHE = {}


def _build():
    nc = bacc.Bacc(
        "TRN2",
        target_bir_lowering=False,
        debug=False,
        num_devices=NCORES,
    )

    wblob = nc.dram_tensor("wblob", [128, WCOLS], F32, kind="ExternalInput").ap()
    protT = nc.dram_tensor("protT", [RES_DIM, L], F32, kind="ExternalInput").ap()
    out_d = nc.dram_tensor("out", [1, 1], F32, kind="ExternalOutput").ap()

    DT = RES_DIM // 128  # 8 k-tiles over the protein feature dim

    with tile.TileContext(nc) as tc:
        with (
            tc.tile_pool(name="const", bufs=1) as cpool,
            tc.tile_pool(name="work", bufs=1) as wpool,
            tc.tile_pool(name="wide", bufs=2) as widepool,
            tc.tile_pool(name="psum", bufs=2, space="PSUM") as psum,
            tc.tile_pool(name="psumY", bufs=1, space="PSUM") as psumY,
        ):
            # ---------------- loads (3 DMAs total) ----------------
            wb = cpool.tile([128, WCOLS], F32, tag="wblob")
            nc.sync.dma_start(wb[:, 0:WSPLIT], wblob[:, 0:WSPLIT])
            nc.sync.dma_start(wb[:, WSPLIT:WCOLS], wblob[:, WSPLIT:WCOLS])

            s_protall = cpool.tile([128, DT * L], F32, tag="prot")
            pt3 = protT.rearrange("(d p) l -> p d l", p=128)
            half = DT // 2
            nc.sync.dma_start(
                s_protall[:, 0 : half * L].rearrange("p (d l) -> p d l", d=half),
                pt3[:, 0:half, :],
            )
            nc.sync.dma_start(
                s_protall[:, half * L : DT * L].rearrange(
                    "p (d l) -> p d l", d=half
                ),
                pt3[:, half:DT, :],
            )

            def wbs(name, rows=slice(0, 128), coff=0, cols=None):
                c0, cw = _WB[name]
                return wb[rows, c0 + coff : c0 + coff + (cols or cw)]

            s_adj = wbs("adj")
            s_eye = wbs("eye")
            s_Wg = wbs("Wg")
            s_WrT = wbs("WrT")
            s_W1t = wbs("W1t")
            s_W1b = wbs("W1b")
            s_W2T = wbs("W2T")
            s_Wn = wbs("Wn", rows=slice(0, 64))
            s_nodesT = wbs("nodesT", rows=slice(0, 64))
            s_Wo = wbs("cols3", coff=0, cols=1)
            s_br = wbs("cols3", coff=1, cols=1)
            s_b2 = wbs("cols3", coff=2, cols=1)
            s_bn = wbs("rowvec", rows=slice(0, 1), coff=0, cols=CG)
            s_bg = wbs("rowvec", rows=slice(0, 1), coff=128, cols=CG)
            s_b1 = wbs("rowvec", rows=slice(0, 1), coff=256, cols=HID)
            s_bo = wbs("rowvec", rows=slice(0, 1), coff=448, cols=1)

            ones1 = cpool.tile([1, 128], F32)
            nc.gpsimd.memset(ones1[:], 1.0)
            ones512 = cpool.tile([1, L], F32)
            nc.gpsimd.memset(ones512[:], 1.0)

            # ---------------- w2o = W2 @ Wo; |w2o| scales + sign columns ----
            ps_w2oa = psum.tile([128, 1], F32, tag="mm")
            nc.tensor.matmul(ps_w2oa[:], s_W2T[:, 0:128], s_Wo[:], start=True, stop=True)
            absw1 = wpool.tile([128, 1], F32)
            nc.scalar.activation(absw1[:], ps_w2oa[:], AF.Abs, scale=SCALE8)
            sgn1 = wpool.tile([128, 1], BF16)
            nc.scalar.activation(sgn1[:], ps_w2oa[:], AF.Sign)
            sgn1_8 = wpool.tile([128, 32], F8)
            nc.gpsimd.memset(sgn1_8[:], 0.0)
            nc.scalar.activation(sgn1_8[:, 0:1], ps_w2oa[:], AF.Sign)
            nc.scalar.activation(sgn1_8[:, 16:17], ps_w2oa[:], AF.Sign)

            s_W2T2r = wpool.tile([CG, 128], F32)
            nc.vector.tensor_copy(s_W2T2r[:, 0:64], s_W2T[:, 128:HID])
            nc.vector.tensor_copy(s_W2T2r[:, 64:128], s_W2T[:, 128:HID])
            ps_w2ob = psum.tile([128, 1], F32, tag="mm")
            nc.tensor.matmul(ps_w2ob[:], s_W2T2r[:], s_Wo[:], start=True, stop=True)
            absw2 = wpool.tile([128, 1], F32)
            nc.scalar.activation(absw2[:], ps_w2ob[:], AF.Abs, scale=SCALE8)
            sgn2 = wpool.tile([128, 1], BF16)
            nc.scalar.activation(sgn2[:], ps_w2ob[:], AF.Sign)
            sgn2_8 = wpool.tile([128, 32], F8)
            nc.gpsimd.memset(sgn2_8[:], 0.0)
            nc.scalar.activation(sgn2_8[:, 0:1], ps_w2ob[:], AF.Sign)
            nc.scalar.activation(sgn2_8[:, 16:17], ps_w2ob[:], AF.Sign)

            # ---------------- adjacency normalization ----------------
            # At = diag(norm) @ A @ diag(norm), norm = clip(deg,1)^-0.5
            deg = wpool.tile([N, 1], F32)
            nc.vector.tensor_reduce(
                deg[:], s_adj[:], axis=mybir.AxisListType.X, op=ALU.add
            )
            nc.vector.tensor_scalar(deg[:], deg[:], 1.0, None, op0=ALU.max)
            sq = wpool.tile([N, 1], F32)
            nc.scalar.activation(sq[:], deg[:], AF.Sqrt)
            norm = wpool.tile([N, 1], F32)
            nc.vector.reciprocal(norm[:], sq[:])

            rowscaled = wpool.tile([N, N], F32)  # norm_i * A_ij
            nc.vector.tensor_scalar(
                rowscaled[:], s_adj[:], norm[:, 0:1], None, op0=ALU.mult
            )
            ps_rsT = psum.tile([N, N], F32, tag="mm")
            nc.tensor.transpose(ps_rsT[:], rowscaled[:], s_eye[:])
            s_At = wpool.tile([N, N], BF16)  # symmetric normalized adjacency
            nc.vector.tensor_scalar(
                s_At[:], ps_rsT[:], norm[:, 0:1], None, op0=ALU.mult
            )
            s_WgBF = wpool.tile([CG, CG], BF16)
            nc.vector.tensor_copy(s_WgBF[:], s_Wg[:])

            # ---------------- GNN ----------------
            # h0 = nodes @ Wn + bn   [N, CG]
            ps_h = psum.tile([N, CG], F32, tag="mm")
            nc.tensor.matmul(ps_h[:], s_nodesT[:], s_Wn[:], start=True, stop=False)
            nc.tensor.matmul(ps_h[:], ones1[:], s_bn[:], start=False, stop=True)
            s_h = wpool.tile([N, CG], BF16, tag="h")
            nc.scalar.activation(s_h[:], ps_h[:], AF.Copy)

            s_h3 = None
            for step in range(NUM_GNN_STEPS):
                last = step == NUM_GNN_STEPS - 1
                # uT = (At @ h)^T = h^T @ At   (At symmetric)  [CG, N]
                ps_uT = psum.tile([CG, N], F32, tag="uT")
                nc.tensor.matmul(ps_uT[:], s_h[:], s_At[:], start=True, stop=True)
                s_uT = wpool.tile([CG, N], BF16, tag="uT_s")
                nc.scalar.activation(s_uT[:], ps_uT[:], AF.Copy)
                # h' = act(u @ Wg + bg)   [N, CG]
                ps_h2 = psum.tile([N, CG], F32, tag="mm")
                nc.tensor.matmul(ps_h2[:], s_uT[:], s_WgBF[:], start=True, stop=False)
                nc.tensor.matmul(ps_h2[:], ones1[:], s_bg[:], start=False, stop=True)
                if last:
                    s_h3 = wpool.tile([N, CG], F32, tag="h3")
                    nc.scalar.activation(s_h3[:], ps_h2[:], AF.Tanh)
                else:
                    s_h = wpool.tile([N, CG], BF16, tag="h")
                    nc.scalar.activation(s_h[:], ps_h2[:], AF.Relu)

            # ---------------- X^T = W1t^T @ h3^T, scaled by |w2o| ----------
            ps_h3T = psum.tile([CG, N], F32, tag="uT")
            nc.tensor.transpose(ps_h3T[:], s_h3[:], s_eye[:])
            s_h3T = wpool.tile([CG, N], F32, tag="uT_s")
            nc.scalar.activation(s_h3T[:], ps_h3T[:], AF.Copy)

            ps_XT1 = psum.tile([128, N], F32, tag="mm")
            nc.tensor.matmul(ps_XT1[:], s_W1t[:, 0:128], s_h3T[:], start=True, stop=True)
            s_XT1 = wpool.tile([128, N], F32)
            nc.scalar.activation(s_XT1[:], ps_XT1[:], AF.Copy, scale=absw1[:, 0:1])

            # chunk-2 biases packed two-per-instruction directly out of PE:
            #   XP2[p<64, j] = X^T[128+p, j]; XP2[p>=64, j] = X^T[128+p-64, 64+j]
            w1t2a = wpool.tile([CG, 128], F32)
            w1t2b = wpool.tile([CG, 128], F32)
            nc.gpsimd.memset(w1t2a[:], 0.0)
            nc.gpsimd.memset(w1t2b[:], 0.0)
            nc.vector.tensor_copy(w1t2a[:, 0:64], s_W1t[:, 128:HID])
            nc.vector.tensor_copy(w1t2b[:, 64:128], s_W1t[:, 128:HID])
            ps_XP2 = psum.tile([128, N // 2], F32, tag="mm")
            nc.tensor.matmul(ps_XP2[:], w1t2a[:], s_h3T[:, 0:64], start=True, stop=False)
            nc.tensor.matmul(
                ps_XP2[:], w1t2b[:], s_h3T[:, 64:128], start=False, stop=True
            )
            s_XP2 = wpool.tile([128, N // 2], F32)
            nc.scalar.activation(s_XP2[:], ps_XP2[:], AF.Copy, scale=absw2[:, 0:1])

            # ---------------- Wc = Wr @ W1b  (-> bf16) ----------------
            s_Wc = []
            for d in range(DT):
                ps_wc = psum.tile([128, HID], F32, tag="mm")
                nc.tensor.matmul(
                    ps_wc[:],
                    s_WrT[:, d * 128 : (d + 1) * 128],
                    s_W1b[:],
                    start=True,
                    stop=True,
                )
                t = wpool.tile([128, HID], BF16, tag=f"wc{d}")
                nc.scalar.activation(t[:], ps_wc[:], AF.Copy)
                s_Wc.append(t)

            # c0 = br @ W1b + b1  (row [1, HID]) folded into Y
            ps_c0 = psum.tile([1, HID], F32, tag="mm")
            nc.tensor.matmul(ps_c0[:], s_br[:], s_W1b[:], start=True, stop=True)
            s_c0 = wpool.tile([1, HID], F32)
            nc.vector.tensor_tensor(s_c0[:], ps_c0[:], s_b1[:], op=ALU.add)

            # chunk-2 stationary tiles with duplicated columns, so the Y2
            # matmul lands already replicated across both partition halves
            s_Wc2r = []
            for d in range(DT):
                t = wpool.tile([128, 128], BF16, tag=f"wc2r{d}")
                nc.vector.tensor_copy(t[:, 0:64], s_Wc[d][:, 128:HID])
                nc.vector.tensor_copy(t[:, 64:128], s_Wc[d][:, 128:HID])
                s_Wc2r.append(t)
            s_c0rep = wpool.tile([1, 128], F32)
            nc.vector.tensor_copy(s_c0rep[:, 0:64], s_c0[:, 128:HID])
            nc.vector.tensor_copy(s_c0rep[:, 64:128], s_c0[:, 128:HID])

            # ---------------- protT -> bf16 ----------------
            s_pbfall = cpool.tile([128, DT * L], BF16, tag="pbf")
            for d in range(DT):
                nc.vector.tensor_copy(
                    s_pbfall[:, d * L : (d + 1) * L],
                    s_protall[:, d * L : (d + 1) * L],
                )

            def pbf(d):
                return s_pbfall[:, d * L : (d + 1) * L]

            # ------- Y^T = Wc^T @ protT + c0, scaled by |w2o|  [HID, L] -----
            ps_Y1 = psumY.tile([128, L], F32, tag="y1")
            ps_Y2 = psumY.tile([128, L], F32, tag="y2")
            for d in range(DT):
                nc.tensor.matmul(
                    ps_Y1[:], s_Wc[d][:, 0:128], pbf(d), start=(d == 0), stop=False
                )
            nc.tensor.matmul(
                ps_Y1[:], s_c0[:, 0:128], ones512[:], start=False, stop=True
            )
            for d in range(DT):
                nc.tensor.matmul(
                    ps_Y2[:], s_Wc2r[d][:], pbf(d), start=(d == 0), stop=False
                )
            nc.tensor.matmul(
                ps_Y2[:], s_c0rep[:], ones512[:], start=False, stop=True
            )

            s_Y1 = wpool.tile([128, L], BF16)
            nc.scalar.activation(s_Y1[:], ps_Y1[:], AF.Copy, scale=absw1[:, 0:1])
            s_Y2r = wpool.tile([128, L], BF16)
            nc.scalar.activation(s_Y2r[:], ps_Y2[:], AF.Copy, scale=absw2[:, 0:1])

            # ---------------- fused relu-sum loop ----------------
            # DVE units: bf16 relu pass (2x) -> wd slot; PE folds each slot as
            #   ps_red[1,512] += sgn_bf16^T @ slot          (512 PE cycles)
            # ACT units: fp8 relu pass -> wa slot; PE folds PAIRS of slots as
            #   ps_red += DoubleRow(sgn_fp8[128,2], [K,2,512]) (256 PE cycles)
            def spread(total, frac):
                k = int(round(total * frac))
                picks = set()
                for i in range(k):
                    picks.add(int(i * total / k))
                return [u in picks for u in range(total)]

            ps_red = psumY.tile([16, L], F32, tag="red")
            nc.vector.memset(ps_red[:], 0.0)
            chunks = [
                (s_Y1, ps_Y1, absw1, s_XT1, sgn1, sgn1_8, N),
                (s_Y2r, ps_Y2, absw2, s_XP2, sgn2, sgn2_8, N // 2),
            ]
            assigns = [spread(nu, ACT_FRAC) for (*_, nu) in chunks]

            # count PE-reduce matmuls: 1 per DVE unit; ceil(k/2) per ACT flush
            # of k slots (flushes happen at G slots or stream end per chunk)
            total_mms = 0
            for (*_2, nunits), on_act in zip(chunks, assigns):
                nact = sum(on_act)
                ndve = nunits - nact
                total_mms += ndve
                full, rem = divmod(nact, G)
                total_mms += full * ((G + 1) // 2)
                if rem:
                    total_mms += (rem + 1) // 2
            mm_idx = [0]

            def red_mm(out_ap, *args, **kw):
                nc.tensor.matmul(
                    out_ap,
                    *args,
                    start=(mm_idx[0] == 0),
                    stop=(mm_idx[0] == total_mms - 1),
                    skip_group_check=True,
                    **kw,
                )
                mm_idx[0] += 1

            def flush_dve(widetile, nslots, sgnc):
                for s in range(nslots):
                    red_mm(ps_red[0:1, :], sgnc[:], widetile[:, s * L : (s + 1) * L])

            def flush_act(widetile, nslots, sgnc8):
                s = 0
                while s + 2 <= nslots:
                    rhs = widetile[:, s * L : (s + 2) * L].rearrange(
                        "k (r f) -> k r f", r=2
                    )
                    lhs = sgnc8[:].rearrange("k (r m) -> k r m", r=2)
                    red_mm(
                        ps_red[0:16, :],
                        lhs,
                        rhs,
                        perf_mode=mybir.MatmulPerfMode.DoubleRow,
                    )
                    s += 2
                if s < nslots:
                    red_mm(
                        ps_red[0:1, :],
                        sgnc8[:, 0:1],
                        widetile[:, s * L : (s + 1) * L],
                    )

            for (ytile, ypsum, absc, xtile, sgnc, sgnc8, nunits), on_act in zip(
                chunks, assigns
            ):
                n_act_left = sum(on_act)
                n_dve_left = nunits - n_act_left
                wd = widepool.tile([128, G * L], BF16, tag="wd")
                wa = widepool.tile([128, G * L], F8, tag="wa")
                ds = asl = 0
                for u in range(nunits):
                    if on_act[u]:
                        nc.scalar.activation(
                            wa[:, asl * L : (asl + 1) * L],
                            ypsum[:],
                            AF.Relu,
                            bias=xtile[:, u : u + 1],
                            scale=absc[:, 0:1],
                        )
                        asl += 1
                        n_act_left -= 1
                        if asl == G or n_act_left == 0:
                            flush_act(wa, asl, sgnc8)
                            wa = widepool.tile([128, G * L], F8, tag="wa")
                            asl = 0
                    else:
                        nc.vector.tensor_scalar(
                            wd[:, ds * L : (ds + 1) * L],
                            ytile[:],
                            xtile[:, u : u + 1],
                            0.0,
                            op0=ALU.add,
                            op1=ALU.max,
                        )
                        ds += 1
                        n_dve_left -= 1
                        if ds == G or n_dve_left == 0:
                            flush_dve(wd, ds, sgnc)
                            wd = widepool.tile([128, G * L], BF16, tag="wd")
                            ds = 0
            assert mm_idx[0] == total_mms, (mm_idx[0], total_mms)

            # ---------------- final scalar ----------------
            red_row = wpool.tile([1, L], F32)
            red_sc = wpool.tile([1, 1], F32)
            nc.vector.tensor_scalar(
                red_row[:],
                ps_red[0:1, :],
                1.0 / SCALE8,
                None,
                op0=ALU.mult,
                op1=ALU.add,
                accum_out=red_sc[:, 0:1],
            )

            # b2 * (N*L) folded bias term
            s_b2s = wpool.tile([CG, 1], F32)
            nc.vector.tensor_scalar(
                s_b2s[:], s_b2[:], float(N * L), None, op0=ALU.mult
            )
            ps_out = psum.tile([1, 1], F32, tag="mm")
            nc.tensor.matmul(ps_out[:], s_b2s[:], s_Wo[:], start=True, stop=True)
            bterm = wpool.tile([1, 1], F32)
            nc.vector.tensor_scalar(
                bterm[:], ps_out[:], s_bo[:, 0:1], None, op0=ALU.add
            )

            s_out = wpool.tile([1, 1], F32)
            nc.vector.tensor_tensor(s_out[:], red_sc[:], bterm[:], op=ALU.add)
            nc.sync.dma_start(out_d[:, :], s_out[:])

    nc.compile()
    return nc


def _shard(inputs):
    adj = np.ascontiguousarray(inputs["adj_mats"], np.float32)
    nodes = np.ascontiguousarray(inputs["nodes"], np.float32)
    prot = np.ascontiguousarray(inputs["protein_sequences"], np.float32)
    W1 = np.asarray(inputs["W1"], np.float32)

    base = np.zeros((128, WCOLS), np.float32)

    def put(name, arr, rows=slice(0, 128), coff=0):
        c0, _ = _WB[name]
        arr = np.asarray(arr, np.float32)
        base[rows, c0 + coff : c0 + coff + arr.shape[1]] = arr

    put("eye", np.eye(N, dtype=np.float32))
    put("Wg", inputs["Wg"])
    put("WrT", np.ascontiguousarray(np.asarray(inputs["Wr"], np.float32).T))
    put("W1t", W1[:CG])
    put("W1b", W1[CG:])
    put("W2T", np.ascontiguousarray(np.asarray(inputs["W2"], np.float32).T))
    put("Wn", inputs["Wn"], rows=slice(0, 64))
    put("cols3", np.asarray(inputs["Wo"], np.float32).reshape(CG, 1), coff=0)
    put("cols3", np.asarray(inputs["br"], np.float32).reshape(CP, 1), coff=1)
    put("cols3", np.asarray(inputs["b2"], np.float32).reshape(CG, 1), coff=2)
    put("rowvec", np.asarray(inputs["bn"], np.float32).reshape(1, CG), coff=0)
    put("rowvec", np.asarray(inputs["bg"], np.float32).reshape(1, CG), coff=128)
    put("rowvec", np.asarray(inputs["b1"], np.float32).reshape(1, HID), coff=256)
    put("rowvec", np.asarray(inputs["bo"], np.float32).reshape(1, 1), coff=448)

    in_maps = []
    for b in range(B):
        blob = base.copy()
        c0, _ = _WB["adj"]
        blob[:, c0 : c0 + N] = adj[b]
        c0, _ = _WB["nodesT"]
        blob[0:64, c0 : c0 + N] = nodes[b].T
        in_maps.append(
            {
                "wblob": blob,
                "protT": np.ascontiguousarray(prot[b].T),
            }
        )
    return in_maps


def _ensure_ntff_hook():
    """This container's `antenv` stub lacks axon_hooks; synthesize it from
    trn_boot's ctypes NTFF hook so run_bass_kernel_spmd(trace=True) works."""
    import types

    try:
        from antenv.axon_hooks import get_axon_ntff_profile_hook  # noqa: F401

        return
    except ImportError:
        pass
    try:
        from trn_agent_boot.trn_boot import _ntff_profile_via_ctypes

        hook = _ntff_profile_via_ctypes("/opt/axon/libaxon_pjrt.so")
    except Exception:
        hook = None
    mod = types.ModuleType("antenv.axon_hooks")
    mod._hook = hook
    mod.get_axon_ntff_profile_hook = lambda: mod._hook
    mod.set_axon_ntff_profile_hook = lambda h: setattr(mod, "_hook", h)
    import antenv

    antenv.axon_hooks = mod
    sys.modules["antenv.axon_hooks"] = mod


def _run(inputs, trace=False):
    if "nc" not in _CACHE:
        _CACHE["nc"] = _build()
    nc = _CACHE["nc"]
    if trace:
        _ensure_ntff_hook()
    res = run_bass_kernel_spmd(
        nc, _shard(inputs), core_ids=list(range(NCORES)), trace=trace
    )
    out = np.zeros((B, 1), np.float32)
    for b in range(B):
        out[b, 0] = np.asarray(res.results[b]["out"]).reshape(-1)[0]
    return out, res


def kernel(**inputs) -> np.ndarray:
    out, _ = _run(inputs, trace=False)
    return out


# revision 21
# speedup vs baseline: 1.0590x; 1.0590x over previous
"""Trainium2 Bass kernel for nn_DecomposableAttentionModel.

Math: the reference's softmax is over a size-1 axis, so attn == 1 exactly and

    out[b] = S[b] @ (W2 @ Wo) + N*L*(b2 @ Wo) + bo
    S[b,h] = sum_{n,l} relu(X[b,n,h] + Y[b,l,h])
    X      = gnn(adj, nodes) @ W1[:CG]
    Y      = prot @ (Wr @ W1[CG:]) + (br @ W1[CG:]) + b1

The [B, N*L, CG+CP] concat tensor is never materialized. Since
w*relu(z) = sign(w)*relu(|w|*z), |w2o| is folded into X and Y at PSUM
evacuation (free per-partition scale on the ACT copy) and the PE reduce
uses exact sign(w2o) stationary vectors.

Sharding: data-parallel over batch B=8, one graph per NeuronCore; weights
replicated. No collectives; the host assembles the [8,1] output. All
weights/biases ship as one packed [128, WCOLS] blob -> a single DMA.

Per-core hot loop (192 units: 128 chunk1 n's + 64 packed chunk2 pairs):
every unit is one relu pass (DVE tensor_scalar 2x | ACT activation) of
Y'[128,512]+x'_u into a wide bf16 buffer; PE folds each slot into
ps_red[1,512] via sign(w2o)^T @ slice matmuls accumulated in PSUM.
"""

import sys

if "/opt/trn_rl_repo" not in sys.path:
    sys.path.insert(0, "/opt/trn_rl_repo")

import numpy as np

import concourse.bacc as bacc
import concourse.bass as bass
import concourse.mybir as mybir
import concourse.tile as tile
from concourse.bass_utils import run_bass_kernel_spmd

B, N, NODE_DIM = 8, 128, 64
L, RES_DIM = 512, 1024
CG, CP = 128, 128
HID = CG + CP // 2  # 192
NUM_GNN_STEPS = 3
NCORES = 8

F32 = mybir.dt.float32
BF16 = mybir.dt.bfloat16
F8 = mybir.dt.float8e4
AF = mybir.ActivationFunctionType
ALU = mybir.AluOpType

# fraction of fused-loop units on the scalar (ACT) engine; rest on DVE.
ACT_FRAC = 0.30
G = 4  # wide-buffer slots per flush
SCALE8 = 256.0  # keeps |w2o|-scaled relu values in fp8e4 range

# packed weight-blob column layout: name -> (col_offset, cols)
_WB = {}
_c = 0
for _name, _w in [
    # part 1 (GNN-critical, lands first): adj..rowvec
    ("adj", N), ("eye", N), ("Wg", CG), ("Wn", N), ("nodesT", N),
    ("W1t", HID), ("W2T", HID), ("cols3", 3), ("rowvec", 449),
    # part 2: protein-path weights
    ("WrT", RES_DIM), ("W1b", HID),
]:
    _WB[_name] = (_c, _w)
    _c += _w
WCOLS = _c
WSPLIT = _WB["WrT"][0]  # columns before this land in the first wblob DMA

_CACHE = {}


def _build():
    nc = bacc.Bacc(
        "TRN2",
        target_bir_lowering=False,
        debug=False,
        num_devices=NCORES,
    )

    wblob = nc.dram_tensor("wblob", [128, WCOLS], F32, kind="ExternalInput").ap()
    protT = nc.dram_tensor("protT", [RES_DIM, L], F32, kind="ExternalInput").ap()
    out_d = nc.dram_tensor("out", [1, 1], F32, kind="ExternalOutput").ap()

    DT = RES_DIM // 128  # 8 k-tiles over the protein feature dim

    with tile.TileContext(nc) as tc:
        with (
            tc.tile_pool(name="const", bufs=1) as cpool,
            tc.tile_pool(name="work", bufs=1) as wpool,
            tc.tile_pool(name="wide", bufs=3) as widepool,
            tc.tile_pool(name="psum", bufs=2, space="PSUM") as psum,
            tc.tile_pool(name="psumY", bufs=1, space="PSUM") as psumY,
        ):
            # ---------------- loads (3 DMAs total) ----------------
            wb = cpool.tile([128, WCOLS], F32, tag="wblob")
            nc.sync.dma_start(wb[:, 0:WSPLIT], wblob[:, 0:WSPLIT])
            nc.sync.dma_start(wb[:, WSPLIT:WCOLS], wblob[:, WSPLIT:WCOLS])

            s_protall = cpool.tile([128, DT * L], F32, tag="prot")
            pt3 = protT.rearrange("(d p) l -> p d l", p=128)
            half = DT // 2
            nc.sync.dma_start(
                s_protall[:, 0 : half * L].rearrange("p (d l) -> p d l", d=half),
                pt3[:, 0:half, :],
            )
            nc.sync.dma_start(
                s_protall[:, half * L : DT * L].rearrange(
                    "p (d l) -> p d l", d=half
                ),
                pt3[:, half:DT, :],
            )

            def wbs(name, rows=slice(0, 128), coff=0, cols=None):
                c0, cw = _WB[name]
                return wb[rows, c0 + coff : c0 + coff + (cols or cw)]

            s_adj = wbs("adj")
            s_eye = wbs("eye")
            s_Wg = wbs("Wg")
            s_WrT = wbs("WrT")
            s_W1t = wbs("W1t")
            s_W1b = wbs("W1b")
            s_W2T = wbs("W2T")
            s_Wn = wbs("Wn", rows=slice(0, 64))
            s_nodesT = wbs("nodesT", rows=slice(0, 64))
            s_Wo = wbs("cols3", coff=0, cols=1)
            s_br = wbs("cols3", coff=1, cols=1)
            s_b2 = wbs("cols3", coff=2, cols=1)
            s_bn = wbs("rowvec", rows=slice(0, 1), coff=0, cols=CG)
            s_bg = wbs("rowvec", rows=slice(0, 1), coff=128, cols=CG)
            s_b1 = wbs("rowvec", rows=slice(0, 1), coff=256, cols=HID)
            s_bo = wbs("rowvec", rows=slice(0, 1), coff=448, cols=1)

            ones1 = cpool.tile([1, 128], F32)
            nc.gpsimd.memset(ones1[:], 1.0)
            ones512 = cpool.tile([1, L], F32)
            nc.gpsimd.memset(ones512[:], 1.0)

            # ---------------- w2o = W2 @ Wo; |w2o| scales + sign columns ----
            ps_w2oa = psum.tile([128, 1], F32, tag="mm")
            nc.tensor.matmul(ps_w2oa[:], s_W2T[:, 0:128], s_Wo[:], start=True, stop=True)
            absw1 = wpool.tile([128, 1], F32)
            nc.scalar.activation(absw1[:], ps_w2oa[:], AF.Abs, scale=SCALE8)
            sgn1 = wpool.tile([128, 1], BF16)
            nc.scalar.activation(sgn1[:], ps_w2oa[:], AF.Sign)
            sgn1_8 = wpool.tile([128, 32], F8)
            nc.gpsimd.memset(sgn1_8[:], 0.0)
            nc.scalar.activation(sgn1_8[:, 0:1], ps_w2oa[:], AF.Sign)
            nc.scalar.activation(sgn1_8[:, 16:17], ps_w2oa[:], AF.Sign)

            s_W2T2r = wpool.tile([CG, 128], F32)
            nc.vector.tensor_copy(s_W2T2r[:, 0:64], s_W2T[:, 128:HID])
            nc.vector.tensor_copy(s_W2T2r[:, 64:128], s_W2T[:, 128:HID])
            ps_w2ob = psum.tile([128, 1], F32, tag="mm")
            nc.tensor.matmul(ps_w2ob[:], s_W2T2r[:], s_Wo[:], start=True, stop=True)
            absw2 = wpool.tile([128, 1], F32)
            nc.scalar.activation(absw2[:], ps_w2ob[:], AF.Abs, scale=SCALE8)
            sgn2 = wpool.tile([128, 1], BF16)
            nc.scalar.activation(sgn2[:], ps_w2ob[:], AF.Sign)
            sgn2_8 = wpool.tile([128, 32], F8)
            nc.gpsimd.memset(sgn2_8[:], 0.0)
            nc.scalar.activation(sgn2_8[:, 0:1], ps_w2ob[:], AF.Sign)
            nc.scalar.activation(sgn2_8[:, 16:17], ps_w2ob[:], AF.Sign)

            # ---------------- adjacency normalization ----------------
            # At = diag(norm) @ A @ diag(norm), norm = clip(deg,1)^-0.5
            deg = wpool.tile([N, 1], F32)
            nc.vector.tensor_reduce(
                deg[:], s_adj[:], axis=mybir.AxisListType.X, op=ALU.add
            )
            nc.vector.tensor_scalar(deg[:], deg[:], 1.0, None, op0=ALU.max)
            sq = wpool.tile([N, 1], F32)
            nc.scalar.activation(sq[:], deg[:], AF.Sqrt)
            norm = wpool.tile([N, 1], F32)
            nc.vector.reciprocal(norm[:], sq[:])

            rowscaled = wpool.tile([N, N], F32)  # norm_i * A_ij
            nc.vector.tensor_scalar(
                rowscaled[:], s_adj[:], norm[:, 0:1], None, op0=ALU.mult
            )
            ps_rsT = psum.tile([N, N], F32, tag="mm")
            nc.tensor.transpose(ps_rsT[:], rowscaled[:], s_eye[:])
            s_At = wpool.tile([N, N], BF16)  # symmetric normalized adjacency
            nc.vector.tensor_scalar(
                s_At[:], ps_rsT[:], norm[:, 0:1], None, op0=ALU.mult
            )
            s_WgBF = wpool.tile([CG, CG], BF16)
            nc.vector.tensor_copy(s_WgBF[:], s_Wg[:])

            # ---------------- GNN ----------------
            # h0 = nodes @ Wn + bn   [N, CG]
            ps_h = psum.tile([N, CG], F32, tag="mm")
            nc.tensor.matmul(ps_h[:], s_nodesT[:], s_Wn[:], start=True, stop=False)
            nc.tensor.matmul(ps_h[:], ones1[:], s_bn[:], start=False, stop=True)
            s_h = wpool.tile([N, CG], BF16, tag="h")
            nc.scalar.activation(s_h[:], ps_h[:], AF.Copy)

            s_h3 = None
            for step in range(NUM_GNN_STEPS):
                last = step == NUM_GNN_STEPS - 1
                # uT = (At @ h)^T = h^T @ At   (At symmetric)  [CG, N]
                ps_uT = psum.tile([CG, N], F32, tag="uT")
                nc.tensor.matmul(ps_uT[:], s_h[:], s_At[:], start=True, stop=True)
                s_uT = wpool.tile([CG, N], BF16, tag="uT_s")
                nc.scalar.activation(s_uT[:], ps_uT[:], AF.Copy)
                # h' = act(u @ Wg + bg)   [N, CG]
                ps_h2 = psum.tile([N, CG], F32, tag="mm")
                nc.tensor.matmul(ps_h2[:], s_uT[:], s_WgBF[:], start=True, stop=False)
                nc.tensor.matmul(ps_h2[:], ones1[:], s_bg[:], start=False, stop=True)
                if last:
                    s_h3 = wpool.tile([N, CG], F32, tag="h3")
                    nc.scalar.activation(s_h3[:], ps_h2[:], AF.Tanh)
                else:
                    s_h = wpool.tile([N, CG], BF16, tag="h")
                    nc.scalar.activation(s_h[:], ps_h2[:], AF.Relu)

            # ---------------- X^T = W1t^T @ h3^T, scaled by |w2o| ----------
            ps_h3T = psum.tile([CG, N], F32, tag="uT")
            nc.tensor.transpose(ps_h3T[:], s_h3[:], s_eye[:])
            s_h3T = wpool.tile([CG, N], F32, tag="uT_s")
            nc.scalar.activation(s_h3T[:], ps_h3T[:], AF.Copy)

            ps_XT1 = psum.tile([128, N], F32, tag="mm")
            nc.tensor.matmul(ps_XT1[:], s_W1t[:, 0:128], s_h3T[:], start=True, stop=True)
            s_XT1 = wpool.tile([128, N], F32)
            nc.scalar.activation(s_XT1[:], ps_XT1[:], AF.Copy, scale=absw1[:, 0:1])

            # chunk-2 biases packed two-per-instruction directly out of PE:
            #   XP2[p<64, j] = X^T[128+p, j]; XP2[p>=64, j] = X^T[128+p-64, 64+j]
            w1t2a = wpool.tile([CG, 128], F32)
            w1t2b = wpool.tile([CG, 128], F32)
            nc.gpsimd.memset(w1t2a[:], 0.0)
            nc.gpsimd.memset(w1t2b[:], 0.0)
            nc.vector.tensor_copy(w1t2a[:, 0:64], s_W1t[:, 128:HID])
            nc.vector.tensor_copy(w1t2b[:, 64:128], s_W1t[:, 128:HID])
            ps_XP2 = psum.tile([128, N // 2], F32, tag="mm")
            nc.tensor.matmul(ps_XP2[:], w1t2a[:], s_h3T[:, 0:64], start=True, stop=False)
            nc.tensor.matmul(
                ps_XP2[:], w1t2b[:], s_h3T[:, 64:128], start=False, stop=True
            )
            s_XP2 = wpool.tile([128, N // 2], F32)
            nc.scalar.activation(s_XP2[:], ps_XP2[:], AF.Copy, scale=absw2[:, 0:1])

            # ---------------- Wc = Wr @ W1b  (-> bf16) ----------------
            s_Wc = []
            for d in range(DT):
                ps_wc = psum.tile([128, HID], F32, tag="mm")
                nc.tensor.matmul(
                    ps_wc[:],
                    s_WrT[:, d * 128 : (d + 1) * 128],
                    s_W1b[:],
                    start=True,
                    stop=True,
                )
                t = wpool.tile([128, HID], BF16, tag=f"wc{d}")
                nc.scalar.activation(t[:], ps_wc[:], AF.Copy)
                s_Wc.append(t)

            # c0 = br @ W1b + b1  (row [1, HID]) folded into Y
            ps_c0 = psum.tile([1, HID], F32, tag="mm")
            nc.tensor.matmul(ps_c0[:], s_br[:], s_W1b[:], start=True, stop=True)
            s_c0 = wpool.tile([1, HID], F32)
            nc.vector.tensor_tensor(s_c0[:], ps_c0[:], s_b1[:], op=ALU.add)

            # chunk-2 stationary tiles with duplicated columns, so the Y2
            # matmul lands already replicated across both partition halves
            s_Wc2r = []
            for d in range(DT):
                t = wpool.tile([128, 128], BF16, tag=f"wc2r{d}")
                nc.vector.tensor_copy(t[:, 0:64], s_Wc[d][:, 128:HID])
                nc.vector.tensor_copy(t[:, 64:128], s_Wc[d][:, 128:HID])
                s_Wc2r.append(t)
            s_c0rep = wpool.tile([1, 128], F32)
            nc.vector.tensor_copy(s_c0rep[:, 0:64], s_c0[:, 128:HID])
            nc.vector.tensor_copy(s_c0rep[:, 64:128], s_c0[:, 128:HID])

            # ---------------- protT -> bf16 ----------------
            s_pbfall = cpool.tile([128, DT * L], BF16, tag="pbf")
            for d in range(DT):
                nc.vector.tensor_copy(
                    s_pbfall[:, d * L : (d + 1) * L],
                    s_protall[:, d * L : (d + 1) * L],
                )

            def pbf(d):
                return s_pbfall[:, d * L : (d + 1) * L]

            # ------- Y^T = Wc^T @ protT + c0, scaled by |w2o|  [HID, L] -----
            ps_Y1 = psumY.tile([128, L], F32, tag="y1")
            ps_Y2 = psumY.tile([128, L], F32, tag="y2")
            for d in range(DT):
                nc.tensor.matmul(
                    ps_Y1[:], s_Wc[d][:, 0:128], pbf(d), start=(d == 0), stop=False
                )
            nc.tensor.matmul(
                ps_Y1[:], s_c0[:, 0:128], ones512[:], start=False, stop=True
            )
            for d in range(DT):
                nc.tensor.matmul(
                    ps_Y2[:], s_Wc2r[d][:], pbf(d), start=(d == 0), stop=False
                )
            nc.tensor.matmul(
                ps_Y2[:], s_c0rep[:], ones512[:], start=False, stop=True
            )

            s_Y1 = wpool.tile([128, L], BF16)
            nc.scalar.activation(s_Y1[:], ps_Y1[:], AF.Copy, scale=absw1[:, 0:1])
            s_Y2r = wpool.tile([128, L], BF16)
            nc.scalar.activation(s_Y2r[:], ps_Y2[:], AF.Copy, scale=absw2[:, 0:1])

            # ---------------- fused relu-sum loop ----------------
            # DVE units: bf16 relu pass (2x) -> wd slot; PE folds each slot as
            #   ps_red[1,512] += sgn_bf16^T @ slot          (512 PE cycles)
            # ACT units: fp8 relu pass -> wa slot; PE folds PAIRS of slots as
            #   ps_red += DoubleRow(sgn_fp8[128,2], [K,2,512]) (256 PE cycles)
            def spread(total, frac):
                k = int(round(total * frac))
                picks = set()
                for i in range(k):
                    picks.add(int(i * total / k))
                return [u in picks for u in range(total)]

            ps_red = psumY.tile([16, L], F32, tag="red")
            nc.vector.memset(ps_red[:], 0.0)
            chunks = [
                (s_Y1, ps_Y1, absw1, s_XT1, sgn1, sgn1_8, N),
                (s_Y2r, ps_Y2, absw2, s_XP2, sgn2, sgn2_8, N // 2),
            ]
            assigns = [spread(nu, ACT_FRAC) for (*_, nu) in chunks]

            # count PE-reduce matmuls: 1 per DVE unit; ceil(k/2) per ACT flush
            # of k slots (flushes happen at G slots or stream end per chunk)
            total_mms = 0
            for (*_2, nunits), on_act in zip(chunks, assigns):
                nact = sum(on_act)
                ndve = nunits - nact
                total_mms += ndve
                full, rem = divmod(nact, G)
                total_mms += full * ((G + 1) // 2)
                if rem:
                    total_mms += (rem + 1) // 2
            mm_idx = [0]

            def red_mm(out_ap, *args, **kw):
                nc.tensor.matmul(
                    out_ap,
                    *args,
                    start=(mm_idx[0] == 0),
                    stop=(mm_idx[0] == total_mms - 1),
                    skip_group_check=True,
                    **kw,
                )
                mm_idx[0] += 1

            def flush_dve(widetile, nslots, sgnc):
                for s in range(nslots):
                    red_mm(ps_red[0:1, :], sgnc[:], widetile[:, s * L : (s + 1) * L])

            def flush_act(widetile, nslots, sgnc8):
                s = 0
                while s + 2 <= nslots:
                    rhs = widetile[:, s * L : (s + 2) * L].rearrange(
                        "k (r f) -> k r f", r=2
                    )
                    lhs = sgnc8[:].rearrange("k (r m) -> k r m", r=2)
                    red_mm(
                        ps_red[0:16, :],
                        lhs,
                        rhs,
                        perf_mode=mybir.MatmulPerfMode.DoubleRow,
                    )
                    s += 2
                if s < nslots:
                    red_mm(
                        ps_red[0:1, :],
                        sgnc8[:, 0:1],
                        widetile[:, s * L : (s + 1) * L],
                    )

            for (ytile, ypsum, absc, xtile, sgnc, sgnc8, nunits), on_act in zip(
                chunks, assigns
            ):
                n_act_left = sum(on_act)
                n_dve_left = nunits - n_act_left
                wd = widepool.tile([128, G * L], BF16, tag="wd")
                wa = widepool.tile([128, G * L], F8, tag="wa")
                ds = asl = 0
                for u in range(nunits):
                    if on_act[u]:
                        nc.scalar.activation(
                            wa[:, asl * L : (asl + 1) * L],
                            ytile[:],
                            AF.Relu,
                            bias=xtile[:, u : u + 1],
                        )
                        asl += 1
                        n_act_left -= 1
                        if asl == G or n_act_left == 0:
                            flush_act(wa, asl, sgnc8)
                            wa = widepool.tile([128, G * L], F8, tag="wa")
                            asl = 0
                    else:
                        nc.vector.tensor_scalar(
                            wd[:, ds * L : (ds + 1) * L],
                            ytile[:],
                            xtile[:, u : u + 1],
                            0.0,
                            op0=ALU.add,
                            op1=ALU.max,
                        )
                        ds += 1
                        n_dve_left -= 1
                        if ds == G or n_dve_left == 0:
                            flush_dve(wd, ds, sgnc)
                            wd = widepool.tile([128, G * L], BF16, tag="wd")
                            ds = 0
            assert mm_idx[0] == total_mms, (mm_idx[0], total_mms)

            # ---------------- final scalar ----------------
            red_row = wpool.tile([1, L], F32)
            red_sc = wpool.tile([1, 1], F32)
            nc.vector.tensor_scalar(
                red_row[:],
                ps_red[0:1, :],
                1.0 / SCALE8,
                None,
                op0=ALU.mult,
                op1=ALU.add,
                accum_out=red_sc[:, 0:1],
            )

            # b2 * (N*L) folded bias term
            s_b2s = wpool.tile([CG, 1], F32)
            nc.vector.tensor_scalar(
                s_b2s[:], s_b2[:], float(N * L), None, op0=ALU.mult
            )
            ps_out = psum.tile([1, 1], F32, tag="mm")
            nc.tensor.matmul(ps_out[:], s_b2s[:], s_Wo[:], start=True, stop=True)
            bterm = wpool.tile([1, 1], F32)
            nc.vector.tensor_scalar(
                bterm[:], ps_out[:], s_bo[:, 0:1], None, op0=ALU.add
            )

            s_out = wpool.tile([1, 1], F32)
            nc.vector.tensor_tensor(s_out[:], red_sc[:], bterm[:], op=ALU.add)
            nc.sync.dma_start(out_d[:, :], s_out[:])

    nc.compile()
    return nc


def _shard(inputs):
    adj = np.ascontiguousarray(inputs["adj_mats"], np.float32)
    nodes = np.ascontiguousarray(inputs["nodes"], np.float32)
    prot = np.ascontiguousarray(inputs["protein_sequences"], np.float32)
    W1 = np.asarray(inputs["W1"], np.float32)

    base = np.zeros((128, WCOLS), np.float32)

    def put(name, arr, rows=slice(0, 128), coff=0):
        c0, _ = _WB[name]
        arr = np.asarray(arr, np.float32)
        base[rows, c0 + coff : c0 + coff + arr.shape[1]] = arr

    put("eye", np.eye(N, dtype=np.float32))
    put("Wg", inputs["Wg"])
    put("WrT", np.ascontiguousarray(np.asarray(inputs["Wr"], np.float32).T))
    put("W1t", W1[:CG])
    put("W1b", W1[CG:])
    put("W2T", np.ascontiguousarray(np.asarray(inputs["W2"], np.float32).T))
    put("Wn", inputs["Wn"], rows=slice(0, 64))
    put("cols3", np.asarray(inputs["Wo"], np.float32).reshape(CG, 1), coff=0)
    put("cols3", np.asarray(inputs["br"], np.float32).reshape(CP, 1), coff=1)
    put("cols3", np.asarray(inputs["b2"], np.float32).reshape(CG, 1), coff=2)
    put("rowvec", np.asarray(inputs["bn"], np.float32).reshape(1, CG), coff=0)
    put("rowvec", np.asarray(inputs["bg"], np.float32).reshape(1, CG), coff=128)
    put("rowvec", np.asarray(inputs["b1"], np.float32).reshape(1, HID), coff=256)
    put("rowvec", np.asarray(inputs["bo"], np.float32).reshape(1, 1), coff=448)

    in_maps = []
    for b in range(B):
        blob = base.copy()
        c0, _ = _WB["adj"]
        blob[:, c0 : c0 + N] = adj[b]
        c0, _ = _WB["nodesT"]
        blob[0:64, c0 : c0 + N] = nodes[b].T
        in_maps.append(
            {
                "wblob": blob,
                "protT": np.ascontiguousarray(prot[b].T),
            }
        )
    return in_maps


def _ensure_ntff_hook():
    """This container's `antenv` stub lacks axon_hooks; synthesize it from
    trn_boot's ctypes NTFF hook so run_bass_kernel_spmd(trace=True) works."""
    import types

    try:
        from antenv.axon_hooks import get_axon_ntff_profile_hook  # noqa: F401

        return
    except ImportError:
        pass
    try:
        from trn_agent_boot.trn_boot import _ntff_profile_via_ctypes

        hook = _ntff_profile_via_ctypes("/opt/axon/libaxon_pjrt.so")
    except Exception:
        hook = None
    mod = types.ModuleType("antenv.axon_hooks")
    mod._hook = hook
    mod.get_axon_ntff_profile_hook = lambda: mod._hook
    mod.set_axon_ntff_profile_hook = lambda h: setattr(mod, "_hook", h)
    import antenv

    antenv.axon_hooks = mod
    sys.modules["antenv.axon_hooks"] = mod


def _run(inputs, trace=False):
    if "nc" not in _CACHE:
        _CACHE["nc"] = _build()
    nc = _CACHE["nc"]
    if trace:
        _ensure_ntff_hook()
    res = run_bass_kernel_spmd(
        nc, _shard(inputs), core_ids=list(range(NCORES)), trace=trace
    )
    out = np.zeros((B, 1), np.float32)
    for b in range(B):
        out[b, 0] = np.asarray(res.results[b]["out"]).reshape(-1)[0]
    return out, res


def kernel(**inputs) -> np.ndarray:
    out, _ = _run(inputs, trace=False)
    return out


# revision 22
# speedup vs baseline: 1.0779x; 1.0179x over previous
"""Trainium2 Bass kernel for nn_DecomposableAttentionModel.

Math: the reference's softmax is over a size-1 axis, so attn == 1 exactly and

    out[b] = S[b] @ (W2 @ Wo) + N*L*(b2 @ Wo) + bo
    S[b,h] = sum_{n,l} relu(X[b,n,h] + Y[b,l,h])
    X      = gnn(adj, nodes) @ W1[:CG]
    Y      = prot @ (Wr @ W1[CG:]) + (br @ W1[CG:]) + b1

The [B, N*L, CG+CP] concat tensor is never materialized. Since
w*relu(z) = sign(w)*relu(|w|*z), |w2o| is folded into X and Y at PSUM
evacuation (free per-partition scale on the ACT copy) and the PE reduce
uses exact sign(w2o) stationary vectors.

Sharding: data-parallel over batch B=8, one graph per NeuronCore; weights
replicated. No collectives; the host assembles the [8,1] output. All
weights/biases ship as one packed [128, WCOLS] blob -> a single DMA.

Per-core hot loop (192 units: 128 chunk1 n's + 64 packed chunk2 pairs):
every unit is one relu pass of Y'[128,512]+x'_u into a wide buffer
(DVE tensor_scalar 2x -> bf16 slots | ACT activation -> fp8e4 slots);
the tensor engine folds slots into ps_red via sign(w2o)^T @ slice
matmuls accumulated in PSUM -- fp8 slot-pairs use a DoubleRow matmul at
0.5 cycles/row. Measured steady-state rates: DVE 263ns/unit, ACT
613ns/unit, PE 216ns/matmul; ~83us end-to-end on silicon.
"""

import sys

if "/opt/trn_rl_repo" not in sys.path:
    sys.path.insert(0, "/opt/trn_rl_repo")

import numpy as np

import concourse.bacc as bacc
import concourse.bass as bass
import concourse.mybir as mybir
import concourse.tile as tile
from concourse.bass_utils import run_bass_kernel_spmd

B, N, NODE_DIM = 8, 128, 64
L, RES_DIM = 512, 1024
CG, CP = 128, 128
HID = CG + CP // 2  # 192
NUM_GNN_STEPS = 3
NCORES = 8

F32 = mybir.dt.float32
BF16 = mybir.dt.bfloat16
F8 = mybir.dt.float8e4
AF = mybir.ActivationFunctionType
ALU = mybir.AluOpType

# fraction of fused-loop units on the scalar (ACT) engine; rest on DVE.
ACT_FRAC = 0.30
G = 4  # wide-buffer slots per flush
SCALE8 = 256.0  # keeps |w2o|-scaled relu values in fp8e4 range

# packed weight-blob column layout: name -> (col_offset, cols)
_WB = {}
_c = 0
for _name, _w in [
    # part 1 (GNN-critical, lands first): adj..rowvec
    ("adj", N), ("eye", N), ("Wg", CG), ("Wn", N), ("nodesT", N),
    ("W1t", HID), ("W2T", HID), ("cols3", 3), ("rowvec", 449),
    # part 2: protein-path weights
    ("WrT", RES_DIM), ("W1b", HID),
]:
    _WB[_name] = (_c, _w)
    _c += _w
WCOLS = _c
WSPLIT = _WB["WrT"][0]  # columns before this land in the first wblob DMA

_CACHE = {}


def _build():
    nc = bacc.Bacc(
        "TRN2",
        target_bir_lowering=False,
        debug=False,
        num_devices=NCORES,
    )

    wblob = nc.dram_tensor("wblob", [128, WCOLS], F32, kind="ExternalInput").ap()
    protT = nc.dram_tensor("protT", [RES_DIM, L], F32, kind="ExternalInput").ap()
    out_d = nc.dram_tensor("out", [1, 1], F32, kind="ExternalOutput").ap()

    DT = RES_DIM // 128  # 8 k-tiles over the protein feature dim

    with tile.TileContext(nc) as tc:
        with (
            tc.tile_pool(name="const", bufs=1) as cpool,
            tc.tile_pool(name="work", bufs=1) as wpool,
            tc.tile_pool(name="wide", bufs=3) as widepool,
            tc.tile_pool(name="psum", bufs=2, space="PSUM") as psum,
            tc.tile_pool(name="psumY", bufs=1, space="PSUM") as psumY,
        ):
            # ---------------- loads (3 DMAs total) ----------------
            wb = cpool.tile([128, WCOLS], F32, tag="wblob")
            nc.sync.dma_start(wb[:, 0:WSPLIT], wblob[:, 0:WSPLIT])
            nc.sync.dma_start(wb[:, WSPLIT:WCOLS], wblob[:, WSPLIT:WCOLS])

            s_protall = cpool.tile([128, DT * L], F32, tag="prot")
            pt3 = protT.rearrange("(d p) l -> p d l", p=128)
            half = DT // 2
            nc.sync.dma_start(
                s_protall[:, 0 : half * L].rearrange("p (d l) -> p d l", d=half),
                pt3[:, 0:half, :],
            )
            nc.sync.dma_start(
                s_protall[:, half * L : DT * L].rearrange(
                    "p (d l) -> p d l", d=half
                ),
                pt3[:, half:DT, :],
            )

            def wbs(name, rows=slice(0, 128), coff=0, cols=None):
                c0, cw = _WB[name]
                return wb[rows, c0 + coff : c0 + coff + (cols or cw)]

            s_adj = wbs("adj")
            s_eye = wbs("eye")
            s_Wg = wbs("Wg")
            s_WrT = wbs("WrT")
            s_W1t = wbs("W1t")
            s_W1b = wbs("W1b")
            s_W2T = wbs("W2T")
            s_Wn = wbs("Wn", rows=slice(0, 64))
            s_nodesT = wbs("nodesT", rows=slice(0, 64))
            s_Wo = wbs("cols3", coff=0, cols=1)
            s_br = wbs("cols3", coff=1, cols=1)
            s_b2 = wbs("cols3", coff=2, cols=1)
            s_bn = wbs("rowvec", rows=slice(0, 1), coff=0, cols=CG)
            s_bg = wbs("rowvec", rows=slice(0, 1), coff=128, cols=CG)
            s_b1 = wbs("rowvec", rows=slice(0, 1), coff=256, cols=HID)
            s_bo = wbs("rowvec", rows=slice(0, 1), coff=448, cols=1)

            ones1 = cpool.tile([1, 128], F32)
            nc.gpsimd.memset(ones1[:], 1.0)
            ones512 = cpool.tile([1, L], F32)
            nc.gpsimd.memset(ones512[:], 1.0)

            # ---------------- w2o = W2 @ Wo; |w2o| scales + sign columns ----
            ps_w2oa = psum.tile([128, 1], F32, tag="mm")
            nc.tensor.matmul(ps_w2oa[:], s_W2T[:, 0:128], s_Wo[:], start=True, stop=True)
            absw1 = wpool.tile([128, 1], F32)
            nc.scalar.activation(absw1[:], ps_w2oa[:], AF.Abs, scale=SCALE8)
            sgn1 = wpool.tile([128, 1], BF16)
            nc.scalar.activation(sgn1[:], ps_w2oa[:], AF.Sign)
            sgn1_8 = wpool.tile([128, 32], F8)
            nc.gpsimd.memset(sgn1_8[:], 0.0)
            nc.scalar.activation(sgn1_8[:, 0:1], ps_w2oa[:], AF.Sign)
            nc.scalar.activation(sgn1_8[:, 16:17], ps_w2oa[:], AF.Sign)

            s_W2T2r = wpool.tile([CG, 128], F32)
            nc.vector.tensor_copy(s_W2T2r[:, 0:64], s_W2T[:, 128:HID])
            nc.vector.tensor_copy(s_W2T2r[:, 64:128], s_W2T[:, 128:HID])
            ps_w2ob = psum.tile([128, 1], F32, tag="mm")
            nc.tensor.matmul(ps_w2ob[:], s_W2T2r[:], s_Wo[:], start=True, stop=True)
            absw2 = wpool.tile([128, 1], F32)
            nc.scalar.activation(absw2[:], ps_w2ob[:], AF.Abs, scale=SCALE8)
            sgn2 = wpool.tile([128, 1], BF16)
            nc.scalar.activation(sgn2[:], ps_w2ob[:], AF.Sign)
            sgn2_8 = wpool.tile([128, 32], F8)
            nc.gpsimd.memset(sgn2_8[:], 0.0)
            nc.scalar.activation(sgn2_8[:, 0:1], ps_w2ob[:], AF.Sign)
            nc.scalar.activation(sgn2_8[:, 16:17], ps_w2ob[:], AF.Sign)

            # ---------------- adjacency normalization ----------------
            # At = diag(norm) @ A @ diag(norm), norm = clip(deg,1)^-0.5
            deg = wpool.tile([N, 1], F32)
            nc.vector.tensor_reduce(
                deg[:], s_adj[:], axis=mybir.AxisListType.X, op=ALU.add
            )
            nc.vector.tensor_scalar(deg[:], deg[:], 1.0, None, op0=ALU.max)
            sq = wpool.tile([N, 1], F32)
            nc.scalar.activation(sq[:], deg[:], AF.Sqrt)
            norm = wpool.tile([N, 1], F32)
            nc.vector.reciprocal(norm[:], sq[:])

            rowscaled = wpool.tile([N, N], F32)  # norm_i * A_ij
            nc.vector.tensor_scalar(
                rowscaled[:], s_adj[:], norm[:, 0:1], None, op0=ALU.mult
            )
            ps_rsT = psum.tile([N, N], F32, tag="mm")
            nc.tensor.transpose(ps_rsT[:], rowscaled[:], s_eye[:])
            s_At = wpool.tile([N, N], BF16)  # symmetric normalized adjacency
            nc.vector.tensor_scalar(
                s_At[:], ps_rsT[:], norm[:, 0:1], None, op0=ALU.mult
            )
            s_WgBF = wpool.tile([CG, CG], BF16)
            nc.vector.tensor_copy(s_WgBF[:], s_Wg[:])

            # ---------------- GNN ----------------
            # h0 = nodes @ Wn + bn   [N, CG]
            ps_h = psum.tile([N, CG], F32, tag="mm")
            nc.tensor.matmul(ps_h[:], s_nodesT[:], s_Wn[:], start=True, stop=False)
            nc.tensor.matmul(ps_h[:], ones1[:], s_bn[:], start=False, stop=True)
            s_h = wpool.tile([N, CG], BF16, tag="h")
            nc.scalar.activation(s_h[:], ps_h[:], AF.Copy)

            s_h3 = None
            for step in range(NUM_GNN_STEPS):
                last = step == NUM_GNN_STEPS - 1
                # uT = (At @ h)^T = h^T @ At   (At symmetric)  [CG, N]
                ps_uT = psum.tile([CG, N], F32, tag="uT")
                nc.tensor.matmul(ps_uT[:], s_h[:], s_At[:], start=True, stop=True)
                s_uT = wpool.tile([CG, N], BF16, tag="uT_s")
                nc.scalar.activation(s_uT[:], ps_uT[:], AF.Copy)
                # h' = act(u @ Wg + bg)   [N, CG]
                ps_h2 = psum.tile([N, CG], F32, tag="mm")
                nc.tensor.matmul(ps_h2[:], s_uT[:], s_WgBF[:], start=True, stop=False)
                nc.tensor.matmul(ps_h2[:], ones1[:], s_bg[:], start=False, stop=True)
                if last:
                    s_h3 = wpool.tile([N, CG], F32, tag="h3")
                    nc.scalar.activation(s_h3[:], ps_h2[:], AF.Tanh)
                else:
                    s_h = wpool.tile([N, CG], BF16, tag="h")
                    nc.scalar.activation(s_h[:], ps_h2[:], AF.Relu)

            # ---------------- X^T = W1t^T @ h3^T, scaled by |w2o| ----------
            ps_h3T = psum.tile([CG, N], F32, tag="uT")
            nc.tensor.transpose(ps_h3T[:], s_h3[:], s_eye[:])
            s_h3T = wpool.tile([CG, N], F32, tag="uT_s")
            nc.scalar.activation(s_h3T[:], ps_h3T[:], AF.Copy)

            ps_XT1 = psum.tile([128, N], F32, tag="mm")
            nc.tensor.matmul(ps_XT1[:], s_W1t[:, 0:128], s_h3T[:], start=True, stop=True)
            s_XT1 = wpool.tile([128, N], F32)
            nc.scalar.activation(s_XT1[:], ps_XT1[:], AF.Copy, scale=absw1[:, 0:1])

            # chunk-2 biases packed two-per-instruction directly out of PE:
            #   XP2[p<64, j] = X^T[128+p, j]; XP2[p>=64, j] = X^T[128+p-64, 64+j]
            w1t2a = wpool.tile([CG, 128], F32)
            w1t2b = wpool.tile([CG, 128], F32)
            nc.gpsimd.memset(w1t2a[:], 0.0)
            nc.gpsimd.memset(w1t2b[:], 0.0)
            nc.vector.tensor_copy(w1t2a[:, 0:64], s_W1t[:, 128:HID])
            nc.vector.tensor_copy(w1t2b[:, 64:128], s_W1t[:, 128:HID])
            ps_XP2 = psum.tile([128, N // 2], F32, tag="mm")
            nc.tensor.matmul(ps_XP2[:], w1t2a[:], s_h3T[:, 0:64], start=True, stop=False)
            nc.tensor.matmul(
                ps_XP2[:], w1t2b[:], s_h3T[:, 64:128], start=False, stop=True
            )
            s_XP2 = wpool.tile([128, N // 2], F32)
            nc.scalar.activation(s_XP2[:], ps_XP2[:], AF.Copy, scale=absw2[:, 0:1])

            # ---------------- Wc = Wr @ W1b  (-> bf16) ----------------
            s_Wc = []
            for d in range(DT):
                ps_wc = psum.tile([128, HID], F32, tag="mm")
                nc.tensor.matmul(
                    ps_wc[:],
                    s_WrT[:, d * 128 : (d + 1) * 128],
                    s_W1b[:],
                    start=True,
                    stop=True,
                )
                t = wpool.tile([128, HID], BF16, tag=f"wc{d}")
                nc.scalar.activation(t[:], ps_wc[:], AF.Copy)
                s_Wc.append(t)

            # c0 = br @ W1b + b1  (row [1, HID]) folded into Y
            ps_c0 = psum.tile([1, HID], F32, tag="mm")
            nc.tensor.matmul(ps_c0[:], s_br[:], s_W1b[:], start=True, stop=True)
            s_c0 = wpool.tile([1, HID], F32)
            nc.vector.tensor_tensor(s_c0[:], ps_c0[:], s_b1[:], op=ALU.add)

            # chunk-2 stationary tiles with duplicated columns, so the Y2
            # matmul lands already replicated across both partition halves
            s_Wc2r = []
            for d in range(DT):
                t = wpool.tile([128, 128], BF16, tag=f"wc2r{d}")
                nc.vector.tensor_copy(t[:, 0:64], s_Wc[d][:, 128:HID])
                nc.vector.tensor_copy(t[:, 64:128], s_Wc[d][:, 128:HID])
                s_Wc2r.append(t)
            s_c0rep = wpool.tile([1, 128], F32)
            nc.vector.tensor_copy(s_c0rep[:, 0:64], s_c0[:, 128:HID])
            nc.vector.tensor_copy(s_c0rep[:, 64:128], s_c0[:, 128:HID])

            # ---------------- protT -> bf16 ----------------
            s_pbfall = cpool.tile([128, DT * L], BF16, tag="pbf")
            for d in range(DT):
                nc.vector.tensor_copy(
                    s_pbfall[:, d * L : (d + 1) * L],
                    s_protall[:, d * L : (d + 1) * L],
                )

            def pbf(d):
                return s_pbfall[:, d * L : (d + 1) * L]

            # ------- Y^T = Wc^T @ protT + c0, scaled by |w2o|  [HID, L] -----
            ps_Y1 = psumY.tile([128, L], F32, tag="y1")
            ps_Y2 = psumY.tile([128, L], F32, tag="y2")
            for d in range(DT):
                nc.tensor.matmul(
                    ps_Y1[:], s_Wc[d][:, 0:128], pbf(d), start=(d == 0), stop=False
                )
            nc.tensor.matmul(
                ps_Y1[:], s_c0[:, 0:128], ones512[:], start=False, stop=True
            )
            for d in range(DT):
                nc.tensor.matmul(
                    ps_Y2[:], s_Wc2r[d][:], pbf(d), start=(d == 0), stop=False
                )
            nc.tensor.matmul(
                ps_Y2[:], s_c0rep[:], ones512[:], start=False, stop=True
            )

            s_Y1 = wpool.tile([128, L], BF16)
            nc.scalar.activation(s_Y1[:], ps_Y1[:], AF.Copy, scale=absw1[:, 0:1])
            s_Y2r = wpool.tile([128, L], BF16)
            nc.scalar.activation(s_Y2r[:], ps_Y2[:], AF.Copy, scale=absw2[:, 0:1])

            # ---------------- fused relu-sum loop ----------------
            # DVE units: bf16 relu pass (2x) -> wd slot; PE folds each slot as
            #   ps_red[1,512] += sgn_bf16^T @ slot          (512 PE cycles)
            # ACT units: fp8 relu pass -> wa slot; PE folds PAIRS of slots as
            #   ps_red += DoubleRow(sgn_fp8[128,2], [K,2,512]) (256 PE cycles)
            def spread(total, frac):
                k = int(round(total * frac))
                picks = set()
                for i in range(k):
                    picks.add(int(i * total / k))
                return [u in picks for u in range(total)]

            ps_red = psumY.tile([16, L], F32, tag="red")
            nc.vector.memset(ps_red[:], 0.0)
            chunks = [
                (s_Y1, ps_Y1, absw1, s_XT1, sgn1, sgn1_8, N),
                (s_Y2r, ps_Y2, absw2, s_XP2, sgn2, sgn2_8, N // 2),
            ]
            assigns = [spread(nu, ACT_FRAC) for (*_, nu) in chunks]

            # count PE-reduce matmuls: 1 per DVE unit; ceil(k/2) per ACT flush
            # of k slots (flushes happen at G slots or stream end per chunk)
            total_mms = 0
            for (*_2, nunits), on_act in zip(chunks, assigns):
                nact = sum(on_act)
                ndve = nunits - nact
                total_mms += ndve
                full, rem = divmod(nact, G)
                total_mms += full * ((G + 1) // 2)
                if rem:
                    total_mms += (rem + 1) // 2
            mm_idx = [0]

            def red_mm(out_ap, *args, **kw):
                nc.tensor.matmul(
                    out_ap,
                    *args,
                    start=(mm_idx[0] == 0),
                    stop=(mm_idx[0] == total_mms - 1),
                    skip_group_check=True,
                    **kw,
                )
                mm_idx[0] += 1

            def flush_dve(widetile, nslots, sgnc):
                for s in range(nslots):
                    red_mm(ps_red[0:1, :], sgnc[:], widetile[:, s * L : (s + 1) * L])

            def flush_act(widetile, nslots, sgnc8):
                s = 0
                while s + 2 <= nslots:
                    rhs = widetile[:, s * L : (s + 2) * L].rearrange(
                        "k (r f) -> k r f", r=2
                    )
                    lhs = sgnc8[:].rearrange("k (r m) -> k r m", r=2)
                    red_mm(
                        ps_red[0:16, :],
                        lhs,
                        rhs,
                        perf_mode=mybir.MatmulPerfMode.DoubleRow,
                    )
                    s += 2
                if s < nslots:
                    red_mm(
                        ps_red[0:1, :],
                        sgnc8[:, 0:1],
                        widetile[:, s * L : (s + 1) * L],
                    )

            for (ytile, ypsum, absc, xtile, sgnc, sgnc8, nunits), on_act in zip(
                chunks, assigns
            ):
                n_act_left = sum(on_act)
                n_dve_left = nunits - n_act_left
                wd = widepool.tile([128, G * L], BF16, tag="wd")
                wa = widepool.tile([128, G * L], F8, tag="wa")
                ds = asl = 0
                for u in range(nunits):
                    if on_act[u]:
                        nc.scalar.activation(
                            wa[:, asl * L : (asl + 1) * L],
                            ytile[:],
                            AF.Relu,
                            bias=xtile[:, u : u + 1],
                        )
                        asl += 1
                        n_act_left -= 1
                        if asl == G or n_act_left == 0:
                            flush_act(wa, asl, sgnc8)
                            wa = widepool.tile([128, G * L], F8, tag="wa")
                            asl = 0
                    else:
                        nc.vector.tensor_scalar(
                            wd[:, ds * L : (ds + 1) * L],
                            ytile[:],
                            xtile[:, u : u + 1],
                            0.0,
                            op0=ALU.add,
                            op1=ALU.max,
                        )
                        ds += 1
                        n_dve_left -= 1
                        if ds == G or n_dve_left == 0:
                            flush_dve(wd, ds, sgnc)
                            wd = widepool.tile([128, G * L], BF16, tag="wd")
                            ds = 0
            assert mm_idx[0] == total_mms, (mm_idx[0], total_mms)

            # ---------------- final scalar ----------------
            red_row = wpool.tile([1, L], F32)
            red_sc = wpool.tile([1, 1], F32)
            nc.vector.tensor_scalar(
                red_row[:],
                ps_red[0:1, :],
                1.0 / SCALE8,
                None,
                op0=ALU.mult,
                op1=ALU.add,
                accum_out=red_sc[:, 0:1],
            )

            # b2 * (N*L) folded bias term
            s_b2s = wpool.tile([CG, 1], F32)
            nc.vector.tensor_scalar(
                s_b2s[:], s_b2[:], float(N * L), None, op0=ALU.mult
            )
            ps_out = psum.tile([1, 1], F32, tag="mm")
            nc.tensor.matmul(ps_out[:], s_b2s[:], s_Wo[:], start=True, stop=True)
            bterm = wpool.tile([1, 1], F32)
            nc.vector.tensor_scalar(
                bterm[:], ps_out[:], s_bo[:, 0:1], None, op0=ALU.add
            )

            s_out = wpool.tile([1, 1], F32)
            nc.vector.tensor_tensor(s_out[:], red_sc[:], bterm[:], op=ALU.add)
            nc.sync.dma_start(out_d[:, :], s_out[:])

    nc.compile()
    return nc


def _shard(inputs):
    adj = np.ascontiguousarray(inputs["adj_mats"], np.float32)
    nodes = np.ascontiguousarray(inputs["nodes"], np.float32)
    prot = np.ascontiguousarray(inputs["protein_sequences"], np.float32)
    W1 = np.asarray(inputs["W1"], np.float32)

    base = np.zeros((128, WCOLS), np.float32)

    def put(name, arr, rows=slice(0, 128), coff=0):
        c0, _ = _WB[name]
        arr = np.asarray(arr, np.float32)
        base[rows, c0 + coff : c0 + coff + arr.shape[1]] = arr

    put("eye", np.eye(N, dtype=np.float32))
    put("Wg", inputs["Wg"])
    put("WrT", np.ascontiguousarray(np.asarray(inputs["Wr"], np.float32).T))
    put("W1t", W1[:CG])
    put("W1b", W1[CG:])
    put("W2T", np.ascontiguousarray(np.asarray(inputs["W2"], np.float32).T))
    put("Wn", inputs["Wn"], rows=slice(0, 64))
    put("cols3", np.asarray(inputs["Wo"], np.float32).reshape(CG, 1), coff=0)
    put("cols3", np.asarray(inputs["br"], np.float32).reshape(CP, 1), coff=1)
    put("cols3", np.asarray(inputs["b2"], np.float32).reshape(CG, 1), coff=2)
    put("rowvec", np.asarray(inputs["bn"], np.float32).reshape(1, CG), coff=0)
    put("rowvec", np.asarray(inputs["bg"], np.float32).reshape(1, CG), coff=128)
    put("rowvec", np.asarray(inputs["b1"], np.float32).reshape(1, HID), coff=256)
    put("rowvec", np.asarray(inputs["bo"], np.float32).reshape(1, 1), coff=448)

    in_maps = []
    for b in range(B):
        blob = base.copy()
        c0, _ = _WB["adj"]
        blob[:, c0 : c0 + N] = adj[b]
        c0, _ = _WB["nodesT"]
        blob[0:64, c0 : c0 + N] = nodes[b].T
        in_maps.append(
            {
                "wblob": blob,
                "protT": np.ascontiguousarray(prot[b].T),
            }
        )
    return in_maps


def _ensure_ntff_hook():
    """This container's `antenv` stub lacks axon_hooks; synthesize it from
    trn_boot's ctypes NTFF hook so run_bass_kernel_spmd(trace=True) works."""
    import types

    try:
        from antenv.axon_hooks import get_axon_ntff_profile_hook  # noqa: F401

        return
    except ImportError:
        pass
    try:
        from trn_agent_boot.trn_boot import _ntff_profile_via_ctypes

        hook = _ntff_profile_via_ctypes("/opt/axon/libaxon_pjrt.so")
    except Exception:
        hook = None
    mod = types.ModuleType("antenv.axon_hooks")
    mod._hook = hook
    mod.get_axon_ntff_profile_hook = lambda: mod._hook
    mod.set_axon_ntff_profile_hook = lambda h: setattr(mod, "_hook", h)
    import antenv

    antenv.axon_hooks = mod
    sys.modules["antenv.axon_hooks"] = mod


def _run(inputs, trace=False):
    if "nc" not in _CACHE:
        _CACHE["nc"] = _build()
    nc = _CACHE["nc"]
    if trace:
        _ensure_ntff_hook()
    res = run_bass_kernel_spmd(
        nc, _shard(inputs), core_ids=list(range(NCORES)), trace=trace
    )
    out = np.zeros((B, 1), np.float32)
    for b in range(B):
        out[b, 0] = np.asarray(res.results[b]["out"]).reshape(-1)[0]
    return out, res


def kernel(**inputs) -> np.ndarray:
    out, _ = _run(inputs, trace=False)
    return out
